# revision 23
# baseline (speedup 1.0000x reference)
"""KoLeo loss kernel for Trainium2 (8 NeuronCores).

loss = -mean_i log( || xn_i - xn_{nn(i)} ||_2 + eps ),  xn = row-normalized x,
nn(i) = argmax_{j != i} xn_i . xn_j.

For unit rows, ||xn_i - xn_j||^2 = 2 - 2 * sim_ij, so only the row MAX of the
similarity matrix (diagonal excluded) is needed, not the argmax.

Distribution: rows are sharded 1024 per core. Each core receives the full
x^T (feature-major) with its columns ROTATED so that the core's own 1024 rows
sit at columns 0..1023 — this makes the program identical across cores
(static diagonal masking), only the data differs.

Per-core device program (cost-model timeline ~267 us, TensorE-bound at 93%;
bf16 matmul roofline for the 8192x8192x1024 gram is 218 us/core):
  stage A: stream x^T fp32 in [128 x 512] tiles (both HWDGE rings); squares
           (ScalarE, bf16 out) -> ones-matmul (TensorE) accumulates column
           norms^2 in fp32 PSUM; sqrt (ScalarE) + reciprocal (VectorE);
           1/norm broadcast along partitions via gpsimd.partition_broadcast;
           normalize fp32 tiles (VectorE mul, single bf16 rounding) -> bf16
           xnt resident in SBUF (16 MB).
  stage B: S-block = xnt_own^T @ xnt (bf16 matmuls, fp32 PSUM accumulate over
           8 k-tiles, 6 PSUM banks deep); add -4 on the static diagonal
           sub-block; row-max of each PSUM tile (VectorE) into a max buffer.
  stage C: s = max over chunks (clamped < 1 for NaN safety);
           log(dist) = 0.5 * ln(2 - 2 s)  [the reference's +eps inside the
           log shifts the result by ~8e-9 absolute - dropped]; the 0.5 is
           folded into the final partition-sum matmul weights (0.5-column).
Host: loss = -(sum of the 8 partials) / 8192.
Measured vs fp32 reference: rel err ~4e-6.
"""

import os
import sys

import numpy as np

for _p in ("/opt/trn_rl_repo", "/root/.axon_site/_ro/trn_rl_repo"):
    if os.path.isdir(_p) and _p not in sys.path:
        sys.path.insert(0, _p)

import ml_dtypes  # noqa: E402
from contextlib import ExitStack  # noqa: E402

import concourse.bass as bass  # noqa: E402
import concourse.tile as tile  # noqa: E402
from concourse import bacc, mybir  # noqa: E402
from concourse.bass_utils import run_bass_kernel_spmd  # noqa: E402

N = 8192          # rows
D = 1024          # features
NCORES = 8
R = N // NCORES   # rows per core (1024)
CH = 512          # column chunk
NCH = N // CH     # 16 chunks
KT = D // 128     # 8 k-tiles (feature tiles of 128)
MT = R // 128     # 8 m-tiles (own-row tiles of 128)
EPS = 1e-8

F32 = mybir.dt.float32
BF16 = mybir.dt.bfloat16
AF = mybir.ActivationFunctionType
AX = mybir.AxisListType

_CACHE = {}


def _build_program():
    nc = bacc.Bacc("TRN2", target_bir_lowering=False, debug=False,
                   num_devices=NCORES)

    xt = nc.dram_tensor("xt", [D, N], F32, kind="ExternalInput").ap()
    losspart = nc.dram_tensor("losspart", [1, 1], F32, kind="ExternalOutput").ap()
    srows = nc.dram_tensor("srows", [128, MT], F32, kind="ExternalOutput").ap()

    negid_np = np.zeros((128, 128), np.float32)
    np.fill_diagonal(negid_np, -4.0)
    negid_d = nc.inline_tensor(negid_np, "negid")
    ones_bf_d = nc.inline_tensor(np.ones((128, 1), ml_dtypes.bfloat16), "ones_bf")
    half_col_d = nc.inline_tensor(np.full((128, 1), 0.5, np.float32), "half_col")
    two_col_d = nc.inline_tensor(np.full((128, 1), 2.0, np.float32), "two_col")

    with tile.TileContext(nc) as tc, ExitStack() as ctx:
        const_pool = ctx.enter_context(tc.tile_pool(name="const", bufs=1))
        xt_pool = ctx.enter_context(tc.tile_pool(name="xtstage", bufs=16))
        sq_pool = ctx.enter_context(tc.tile_pool(name="sq", bufs=4))
        xnt_pool = ctx.enter_context(tc.tile_pool(name="xnt", bufs=1))
        inv_pool = ctx.enter_context(tc.tile_pool(name="inv", bufs=2))
        stat_pool = ctx.enter_context(tc.tile_pool(name="stat", bufs=1))
        ps_norm = ctx.enter_context(tc.tile_pool(name="psnorm", bufs=2, space="PSUM"))
        ps_s = ctx.enter_context(tc.tile_pool(name="psS", bufs=6, space="PSUM"))

        # preload ACT function tables while everything is idle
        pre = stat_pool.tile([128, 3], F32, tag="pre")
        nc.vector.memset(pre[:], 1.0)
        nc.scalar.activation(pre[:, 0:1], pre[:, 0:1], AF.Square)
        nc.scalar.activation(pre[:, 1:2], pre[:, 1:2], AF.Sqrt)
        nc.scalar.activation(pre[:, 2:3], pre[:, 2:3], AF.Ln)

        negid = const_pool.tile([128, 128], F32, tag="negid")
        nc.gpsimd.dma_start(negid[:], negid_d[:, :])
        ones_bf = const_pool.tile([128, 1], BF16, tag="ones_bf")
        nc.gpsimd.dma_start(ones_bf[:], ones_bf_d[:, :])
        half_col = const_pool.tile([128, 1], F32, tag="half_col")
        nc.gpsimd.dma_start(half_col[:], half_col_d[:, :])
        two_col = const_pool.tile([128, 1], F32, tag="two_col")
        nc.gpsimd.dma_start(two_col[:], two_col_d[:, :])

        maxbuf = stat_pool.tile([128, MT * NCH], F32, tag="maxbuf")
        sbuf_s = stat_pool.tile([128, MT], F32, tag="srows")
        logbuf = stat_pool.tile([128, MT], F32, tag="logbuf")

        xnt = [[None] * NCH for _ in range(KT)]

        # ---- stage A: load, norms, normalize to bf16 ----
        for n in range(NCH):
            nsq = ps_norm.tile([1, CH], F32, tag="nsq")
            stg = []
            sqs = []
            for k in range(KT):
                t = xt_pool.tile([128, CH], F32, tag="xstage")
                dma_eng = nc.sync if k % 2 == 0 else nc.scalar
                dma_eng.dma_start(t[:], xt[k * 128:(k + 1) * 128,
                                           n * CH:(n + 1) * CH])
                stg.append(t)
                sq = sq_pool.tile([128, CH], BF16, tag="sq", bufs=6)
                if n == 0 and k % 2 == 1:
                    # first chunk is latency-critical: split squares ACT/DVE
                    nc.vector.tensor_mul(sq[:], t[:], t[:])
                else:
                    nc.scalar.activation(sq[:], t[:], AF.Square)
                nc.tensor.matmul(nsq[:], ones_bf[:], sq[:],
                                 start=(k == 0), stop=(k == KT - 1))
            nrm = inv_pool.tile([1, CH], F32, tag="nrm")
            nc.scalar.activation(nrm[:], nsq[:], AF.Sqrt)
            inv = inv_pool.tile([1, CH], F32, tag="inv")
            nc.vector.reciprocal(inv[:], nrm[:])
            scl = sq_pool.tile([128, CH], F32, tag="scl", bufs=2)
            nc.gpsimd.partition_broadcast(scl[:], inv[:])
            for k in range(KT):
                xx = xnt_pool.tile([128, CH], BF16, tag=f"xnt{k}_{n}")
                nc.vector.tensor_mul(xx[:], stg[k][:], scl[:])
                xnt[k][n] = xx

        # ---- stage B: similarity row-max (+ per-m epilogue on last chunk) ----
        for n in range(NCH):
            for m in range(MT):
                ck, off = m // 4, (m % 4) * 128
                s_ps = ps_s.tile([128, CH], F32)
                for k in range(KT):
                    nc.tensor.matmul(s_ps[:], xnt[k][ck][:, off:off + 128],
                                     xnt[k][n][:],
                                     start=(k == 0), stop=(k == KT - 1))
                if n == ck:
                    nc.vector.tensor_add(s_ps[:, off:off + 128],
                                         s_ps[:, off:off + 128], negid[:])
                col = m * NCH + n
                nc.vector.reduce_max(maxbuf[:, col:col + 1], s_ps[:], axis=AX.X)
                if n == NCH - 1:
                    # stage C for this m: s -> log(dist^2)/2
                    nc.vector.reduce_max(sbuf_s[:, m:m + 1],
                                         maxbuf[:, m * NCH:(m + 1) * NCH],
                                         axis=AX.X)
                    # guard: keep 2 - 2s strictly positive even for
                    # pathological near-duplicate rows (avoids NaN in Ln)
                    nc.vector.tensor_scalar_min(sbuf_s[:, m:m + 1],
                                                sbuf_s[:, m:m + 1],
                                                1.0 - 1e-7)
                    nc.scalar.activation(logbuf[:, m:m + 1], sbuf_s[:, m:m + 1],
                                         AF.Ln, bias=two_col[:], scale=-2.0)

        # ---- stage D: partition-sum of logs -> scalar ----
        fin_full = ps_norm.tile([1, CH], F32, tag="nsq")
        fin = fin_full[:, :MT]
        nc.tensor.matmul(fin[:], half_col[:], logbuf[:], start=True, stop=True)
        tot = stat_pool.tile([1, 1], F32, tag="tot")
        nc.vector.reduce_sum(tot[:], fin[:], axis=AX.X)
        nc.sync.dma_start(losspart[:], tot[:])
        nc.sync.dma_start(srows[:, :], sbuf_s[:])

    nc.compile()
    return nc


def _run(student_output: np.ndarray, **spmd_kwargs):
    x = np.asarray(student_output, dtype=np.float32)
    assert x.shape == (N, D), x.shape

    if "nc" not in _CACHE:
        _CACHE["nc"] = _build_program()
    nc = _CACHE["nc"]

    xtf = np.ascontiguousarray(x.T)  # [D, N]
    in_maps = []
    for c in range(NCORES):
        s = c * R
        rolled = np.concatenate([xtf[:, s:], xtf[:, :s]], axis=1) if s else xtf
        in_maps.append({"xt": np.ascontiguousarray(rolled)})

    res = run_bass_kernel_spmd(nc, in_maps, list(range(NCORES)), **spmd_kwargs)
    total = np.float64(0.0)
    for c in range(NCORES):
        total += np.float64(res.results[c]["losspart"][0, 0])
    return np.asarray(-total / N, dtype=np.float32), res


def kernel(student_output: np.ndarray) -> np.ndarray:
    return _run(student_output)[0]
